# revision 3
# baseline (speedup 1.0000x reference)
"""ProteinInterfacePrediction fused Bass kernel for 8 TRN2 NeuronCores.

Sharding: core c = (batch b = c//2, half h = c%2); each core computes the
(256, 512) output tile for L-rows [256h, 256h+256).

GNN dedupe: within a batch pair, the EVEN core runs the receptor GNN and the
ODD core runs the ligand GNN (full 512 nodes each); the (32,512) HOPI
projections are exchanged on-chip via a pairwise AllGather, so every edge
byte is shipped to the device exactly once.

Decomposition (validated bit-level in numpy vs the jax reference):
  - GNN residual folded into HOPI: proj = Wp@nodeT + (Wp/16)@S, S = sum_k tanh(hn+he)
  - conv1 is rank-separable before relu: conv1(P) = U[co,l] + V[co,r] (+consts),
    boundary columns via mask-augmented 1-D convs, boundary rows via flag-built
    V-weight variants.
  - conv2 on TensorE: 4-input-row blocks on 128 partitions (K = 4rows x 32ci),
    stride-2 (P/Q dual layouts), 3 dr-taps, 4-way 32-column array tiling.
  - conv3 (1x1) + bias + sigmoid fused at the tail.

Wire-format optimizations (the harness metric is wall-clock of
run_bass_kernel_spmd, dominated by host<->device transfer + dispatch):
  - edge features shipped as fp8e4m3, nodes + weights as bf16 (adds ~4e-4
    rel err vs the 2e-2 budget)
  - all small constants packed into 4 tensors (8 inputs/core)
  - output shipped as uint8 (sigmoid * 255; quantization err <= 1/255)
  - persistent jax compilation cache so the per-call jit rebuild inside
    run_bass_via_pjrt hits disk instead of recompiling XLA
"""

import numpy as np
import ml_dtypes

try:  # make the per-call jit re-lowering inside run_bass_via_pjrt cacheable
    import jax as _jax
    _jax.config.update("jax_compilation_cache_dir", "/tmp/jaxcache")
    _jax.config.update("jax_persistent_cache_min_compile_time_secs", 0.0)
    _jax.config.update("jax_persistent_cache_min_entry_size_bytes", -1)
except Exception:
    pass

B, L, R, KNB = 4, 512, 512, 16
DN, DE = 128, 64
NN = 512                 # nodes per GNN (one full molecule per core)
PN = NN * KNB
CH = 64                  # gnn nodes per chunk
NSTRIP = 8

_CACHE = {}

# packed-constant column layouts
_BPK = dict(WNT=(0, 128), WpT=(128, 160), W2P0=(160, 256), W2P1=(256, 352),
            W3selb=(352, 356))
_CPK128 = dict(gnnbias=(0, 1), WpT16=(1, 33), bc2rep=(33, 34), b3vec=(34, 35),
               rmP0=(35, 36), rmQ63=(36, 37), f0col=(37, 38), f1col=(38, 39))
_CPK32 = dict(UW=(0, 96), A0W=(96, 192), A511W=(192, 288), W1c0=(288, 480),
              W1c511=(480, 672), VW=(672, 1056), X0P=(1056, 1152),
              X2P=(1152, 1248))
_CPK1 = dict(c0const=(0, 96), c511const=(96, 192), VC=(192, 320),
             VCfirst=(320, 448), VCqlast=(448, 576), ONE1=(576, 577),
             ONESR=(577, 1089), plmaskrow=(1089, 1349))


def _host_prep(inputs):
    f32 = np.float32
    bf16 = ml_dtypes.bfloat16
    fp8 = ml_dtypes.float8_e4m3  # == mybir.dt.np(dt.float8e4)
    W1 = np.asarray(inputs['Wc1'], f32)
    W2 = np.asarray(inputs['Wc2'], f32)
    W3 = np.asarray(inputs['Wc3'], f32)[0, :, 0, 0]
    b1 = np.asarray(inputs['bc1'], f32)
    b2 = np.asarray(inputs['bc2'], f32)
    b3 = float(np.asarray(inputs['bc3'], f32)[0])
    Wp = np.asarray(inputs['Wp'], f32)
    bp = np.asarray(inputs['bp'], f32)
    Wl, Wr = Wp[:, :DN], Wp[:, DN:]
    WN = np.asarray(inputs['WN'], f32)
    bN = np.asarray(inputs['bN'], f32)
    WE = np.asarray(inputs['WE'], f32)
    bE = np.asarray(inputs['bE'], f32)

    A = W1.sum(axis=3)
    Wv = W1.sum(axis=2)
    cU = np.einsum('oidr,i->od', W1, bp)

    # ---- shared bf16 pack pieces (WpT slot filled per-core) ----
    bpk0 = np.zeros((128, 356), bf16)

    def bput(pk, name, arr):
        a, b_ = _BPK[name]
        pk[:arr.shape[0], a:b_] = arr.astype(bf16)

    bput(bpk0, 'WNT', np.ascontiguousarray(WN.T))
    W2P0 = np.zeros((128, 96), f32)
    W2P1 = np.zeros((128, 96), f32)
    for dr in range(3):
        for j in range(3):
            W2P0[32 * j:32 * j + 32, 32 * dr:32 * dr + 32] = W2[:, :, j, dr].T
        for j in range(1, 4):
            W2P1[32 * j:32 * j + 32, 32 * dr:32 * dr + 32] = W2[:, :, j - 1, dr].T
    bput(bpk0, 'W2P0', W2P0)
    bput(bpk0, 'W2P1', W2P1)
    W3sel = np.zeros((128, 4), f32)
    for j in range(4):
        W3sel[32 * j:32 * j + 32, j] = W3
    bput(bpk0, 'W3selb', W3sel)

    # ---- shared 32-row f32 pieces ----
    def pack3(M):  # (co, ci, dl) -> [32, 96] of [ci, co] blocks
        out = np.zeros((32, 96), f32)
        for dl in range(3):
            out[:, 32 * dl:32 * dl + 32] = M[:, :, dl].T
        return out

    cpk32_0 = np.zeros((32, 1248), f32)

    def c32put(name, arr):
        a, b_ = _CPK32[name]
        cpk32_0[:arr.shape[0], a:b_] = arr

    c32put('UW', pack3(A))
    c32put('A0W', pack3(W1[:, :, :, 1:].sum(axis=3)))
    c32put('A511W', pack3(W1[:, :, :, :2].sum(axis=3)))

    W1c0 = np.zeros((32, 192), f32)
    W1c511 = np.zeros((32, 192), f32)
    for dl in range(3):
        for t, dr in enumerate((1, 2)):
            W1c0[:, 32 * (2 * dl + t):32 * (2 * dl + t) + 32] = W1[:, :, dl, dr].T
        for t, dr in enumerate((0, 1)):
            W1c511[:, 32 * (2 * dl + t):32 * (2 * dl + t) + 32] = W1[:, :, dl, dr].T
    c32put('W1c0', W1c0)
    c32put('W1c511', W1c511)

    VW = np.zeros((32, 384), f32)
    for dr in range(3):
        blk = Wv[:, :, dr].T
        for j in range(4):
            VW[:, 128 * dr + 32 * j:128 * dr + 32 * j + 32] = blk
    c32put('VW', VW)
    X0P = np.zeros((32, 96), f32)
    X2P = np.zeros((32, 96), f32)
    for dr in range(3):
        X0P[:, 32 * dr:32 * dr + 32] = W1[:, :, 0, dr].T
        X2P[:, 32 * dr:32 * dr + 32] = W1[:, :, 2, dr].T
    c32put('X0P', X0P)
    c32put('X2P', X2P)

    # ---- shared 1-row f32 pieces (VCfirst/VCqlast flag-baked per core) ----
    c0c = np.zeros((1, 96), f32)
    c511c = np.zeros((1, 96), f32)
    for dl in range(3):
        c0c[0, 32 * dl:32 * dl + 32] = np.einsum('oid,i->o', W1[:, :, dl, 1:], bp)
        c511c[0, 32 * dl:32 * dl + 32] = np.einsum('oid,i->o', W1[:, :, dl, :2], bp)
    c0c[0, 32:64] += b1
    c511c[0, 32:64] += b1
    vc = cU.sum(axis=1) + b1
    VC = np.tile(vc, 4).reshape(1, 128).astype(f32)

    sh = {'wfp8': np.ascontiguousarray(WE.T).astype(fp8)}

    lig_nf = np.asarray(inputs['ligand_node_features'], f32)
    lig_ef = np.asarray(inputs['ligand_edge_features'], f32)
    rec_nf = np.asarray(inputs['receptor_node_features'], f32)
    rec_ef = np.asarray(inputs['receptor_edge_features'], f32)

    maps = []
    for core in range(8):
        b, h = core // 2, core % 2
        lo = 256 * h - 2
        m = dict(sh)

        # even core: receptor GNN; odd core: ligand GNN
        if h == 0:
            nf, ef, Wpp = rec_nf[b], rec_ef[b], Wr
        else:
            nf, ef, Wpp = lig_nf[b], lig_ef[b], Wl
        m['nodeT'] = np.ascontiguousarray(nf.T).astype(bf16)
        m['edgeT'] = np.ascontiguousarray(ef.reshape(PN, DE).T).astype(fp8)

        bpk = bpk0.copy()
        bput(bpk, 'WpT', np.ascontiguousarray(Wpp.T))
        m['bpk'] = bpk

        cpk128 = np.zeros((128, 39), f32)

        def c128put(name, arr):
            a, b_ = _CPK128[name]
            cpk128[:arr.shape[0], a:b_] = arr

        c128put('gnnbias', (bN + bE).reshape(DN, 1))
        c128put('WpT16', np.ascontiguousarray((Wpp / 16.0).T)[:, 0:32])
        c128put('bc2rep', np.tile(b2, 4).reshape(128, 1))
        c128put('b3vec', np.full((128, 1), b3, f32))
        flag0 = 1.0 if h == 0 else 0.0
        flag1 = 1.0 if h == 1 else 0.0
        c128put('f0col', np.full((128, 1), flag0, f32))
        c128put('f1col', np.full((128, 1), flag1, f32))
        rmP0 = np.ones((128, 1), f32)
        rmQ63 = np.ones((128, 1), f32)
        for j in range(4):
            if not (0 <= 256 * h + (j - 1) < L):
                rmP0[32 * j:32 * j + 32] = 0.0
            if not (0 <= 256 * h + (253 + j) < L):
                rmQ63[32 * j:32 * j + 32] = 0.0
        c128put('rmP0', rmP0)
        c128put('rmQ63', rmQ63)
        m['cpk128'] = cpk128

        m['cpk32'] = cpk32_0

        cpk1 = np.zeros((1, 1349), f32)

        def c1put(name, arr):
            a, b_ = _CPK1[name]
            cpk1[:, a:b_] = arr

        c1put('c0const', c0c)
        c1put('c511const', c511c)
        VCfirst, VCqlast = VC.copy(), VC.copy()
        VCfirst[0, 32:64] -= flag0 * cU[:, 0]
        VCqlast[0, 64:96] -= flag1 * cU[:, 2]
        c1put('VC', VC)
        c1put('VCfirst', VCfirst)
        c1put('VCqlast', VCqlast)
        c1put('ONE1', np.ones((1, 1), f32))
        c1put('ONESR', np.ones((1, 512), f32))
        plmask = np.array([1.0 if 0 <= lo + i < L else 0.0 for i in range(260)],
                          f32)
        c1put('plmaskrow', plmask.reshape(1, 260))
        m['cpk1'] = cpk1
        maps.append(m)
    return maps


def _build_program():
    import concourse.bacc as bacc
    import concourse.mybir as mybir
    from concourse.tile import TileContext

    dt = mybir.dt
    f32, bf16, fp8, u8 = dt.float32, dt.bfloat16, dt.float8e4, dt.uint8
    AF = mybir.ActivationFunctionType
    ALU = mybir.AluOpType

    nc = bacc.Bacc("TRN2", target_bir_lowering=False, debug=False, num_devices=8)

    def din(name, shape, dtype=f32):
        return nc.dram_tensor(name, list(shape), dtype, kind="ExternalInput")

    nodeTd = din("nodeT", (128, NN), bf16)
    edgeTd = din("edgeT", (64, PN), fp8)
    wfp8d = din("wfp8", (64, 128), fp8)
    bpkd = din("bpk", (128, 356), bf16)
    cpk128d = din("cpk128", (128, 39))
    cpk32d = din("cpk32", (32, 1248))
    cpk1d = din("cpk1", (1, 1349))
    out = nc.dram_tensor("out", [512, 256], u8, kind="ExternalOutput")

    with TileContext(nc) as tc:
        with tc.tile_pool(name="const", bufs=1) as cpool, \
             tc.tile_pool(name="dram", bufs=1, space="DRAM") as dpool:
            WETb_s = cpool.tile([128, 128], fp8, tag="wfp8")
            nc.sync.dma_start(out=WETb_s[0:64, :], in_=wfp8d[:])
            bpk_s = cpool.tile([128, 356], bf16, tag="bpk")
            nc.sync.dma_start(out=bpk_s[:], in_=bpkd[:])
            cpk128_s = cpool.tile([128, 39], f32, tag="cpk128")
            nc.sync.dma_start(out=cpk128_s[:], in_=cpk128d[:])
            cpk32_s = cpool.tile([128, 1248], f32, tag="cpk32")
            nc.sync.dma_start(out=cpk32_s[0:32, :], in_=cpk32d[:])
            cpk1_s = cpool.tile([128, 1349], f32, tag="cpk1")
            nc.sync.dma_start(out=cpk1_s[0:1, :], in_=cpk1d[:])
            nodeTb_s = cpool.tile([128, NN], bf16, tag="nodeTb")
            nc.gpsimd.dma_start(out=nodeTb_s[:], in_=nodeTd[:])

            def bsl(name, rows=128):
                a, b_ = _BPK[name]
                return bpk_s[0:rows, a:b_]

            def c128sl(name, rows=128):
                a, b_ = _CPK128[name]
                return cpk128_s[0:rows, a:b_]

            def c32sl(name, rows=32):
                a, b_ = _CPK32[name]
                return cpk32_s[0:rows, a:b_]

            def c1sl(name):
                a, b_ = _CPK1[name]
                return cpk1_s[0:1, a:b_]

            WNT_s = bsl('WNT')
            WpT_s = bsl('WpT')
            W2P0_s, W2P1_s = bsl('W2P0'), bsl('W2P1')
            W3sel_s = bsl('W3selb')
            gnnbias_s = c128sl('gnnbias')
            WpT16_s = c128sl('WpT16')
            bc2rep_s, b3vec_s = c128sl('bc2rep'), c128sl('b3vec')
            rmP0_s, rmQ63_s = c128sl('rmP0'), c128sl('rmQ63')
            f0col_s, f1col_s = c128sl('f0col'), c128sl('f1col')
            UW_s = c32sl('UW')
            W1c0_s, W1c511_s = c32sl('W1c0'), c32sl('W1c511')
            VW_s = c32sl('VW')
            X0P_s, X2P_s = c32sl('X0P'), c32sl('X2P')
            c0c_s, c511c_s = c1sl('c0const'), c1sl('c511const')
            VC_s, VCf_s, VCq_s = c1sl('VC'), c1sl('VCfirst'), c1sl('VCqlast')
            ONE1_s, ONESR_s = c1sl('ONE1'), c1sl('ONESR')
            plmaskrow_s = c1sl('plmaskrow')

            S_t = cpool.tile([128, NN], f32)
            projm = cpool.tile([128, 512], f32)   # rows 0:32: own projection
            plx = cpool.tile([128, 516], f32)     # rows 0:32: [0,0, pl, 0,0]
            selt = cpool.tile([128, 260], f32)
            plzA = cpool.tile([128, 260], f32)    # rows 0-31 plz, row 32 mask
            przA = cpool.tile([128, 514], f32)
            U_sb = cpool.tile([128, 260], f32)
            Uc0_sb = cpool.tile([128, 260], f32)
            Uc511_sb = cpool.tile([128, 260], f32)
            A0AUG = cpool.tile([128, 96], f32)
            A511AUG = cpool.tile([128, 96], f32)
            VWf_t = cpool.tile([128, 384], f32)
            VWq_t = cpool.tile([128, 384], f32)
            Xf_t = cpool.tile([128, 96], f32)
            V_rep = cpool.tile([128, 512], f32)
            V_first = cpool.tile([128, 512], f32)
            V_qlast = cpool.tile([128, 512], f32)
            uP = cpool.tile([128, 64], f32, tag="uP")
            uQ = cpool.tile([128, 64], f32, tag="uQ")
            uc0P = cpool.tile([128, 64], f32, tag="uc0P")
            uc0Q = cpool.tile([128, 64], f32, tag="uc0Q")
            uc511P = cpool.tile([128, 64], f32, tag="uc511P")
            uc511Q = cpool.tile([128, 64], f32, tag="uc511Q")

            # ================= GNN phase (own molecule only) =================
            with tc.tile_pool(name="gnn", bufs=4) as gpool, \
                 tc.tile_pool(name="gpsum", bufs=3, space="PSUM") as gpsum, \
                 tc.tile_pool(name="spsum", bufs=1, space="PSUM") as spsum:

                for c in range(PN // (CH * KNB)):
                    et = gpool.tile([128, CH * KNB], fp8, tag="edge")
                    nc.gpsimd.dma_start(
                        out=et[0:64, :],
                        in_=edgeTd[:, c * CH * KNB:(c + 1) * CH * KNB])
                    hz = gpsum.tile([128, CH * KNB], f32, tag="hz")
                    for q in range(CH * KNB // 512):
                        nc.tensor.matmul(
                            hz[:, q * 512:(q + 1) * 512],
                            WETb_s[0:64, :],
                            et[0:64, q * 512:(q + 1) * 512],
                            start=True, stop=False)
                        rhs = nodeTb_s[:, c * CH + q * 32:c * CH + (q + 1) * 32]
                        rhs = rhs.unsqueeze(2).broadcast_to([128, 32, KNB])
                        nc.tensor.matmul(
                            hz[:, q * 512:(q + 1) * 512],
                            WNT_s, rhs,
                            start=False, stop=True)
                    zt = gpool.tile([128, CH * KNB], bf16, tag="zt")
                    nc.scalar.activation(zt[:], hz[:], AF.Tanh, bias=gnnbias_s)
                    ztr = zt[:].rearrange("p (n k) -> p n k", k=KNB)
                    nc.vector.reduce_sum(
                        S_t[:, c * CH:(c + 1) * CH], ztr,
                        axis=mybir.AxisListType.X)

                # ---- HOPI projection of own molecule ----
                pp = spsum.tile([128, 512], f32, tag="sp")
                nc.tensor.matmul(pp[0:32, 0:NN], WpT_s, nodeTb_s[:],
                                 start=True, stop=False)
                nc.tensor.matmul(pp[0:32, 0:NN], WpT16_s, S_t[:],
                                 start=False, stop=True)
                nc.scalar.activation(projm[0:32, :], pp[0:32, 0:NN], AF.Copy)

                # ---- pairwise exchange: rows 0:32 = pr (even), 32:64 = pl ----
                cin = dpool.tile([32, 512], f32, tag="cin")
                cout = dpool.tile([64, 512], f32, tag="cout")
                nc.gpsimd.dma_start(out=cin[:], in_=projm[0:32, :])
                nc.gpsimd.collective_compute(
                    "AllGather", ALU.bypass,
                    replica_groups=[[0, 1], [2, 3], [4, 5], [6, 7]],
                    ins=[cin.opt()], outs=[cout.opt()])
                nc.vector.memset(przA[0:32, 0:1], 0.0)
                nc.vector.memset(przA[0:32, 513:514], 0.0)
                nc.sync.dma_start(out=przA[0:32, 1:513], in_=cout[0:32, :])
                nc.vector.memset(plx[0:32, 0:2], 0.0)
                nc.vector.memset(plx[0:32, 514:516], 0.0)
                nc.sync.dma_start(out=plx[0:32, 2:514], in_=cout[32:64, :])

                # ---- plzA = f0*plx[:,0:260] + f1*plx[:,256:516]; row 32 mask ----
                f0c32, f1c32 = c128sl('f0col', rows=32), c128sl('f1col', rows=32)
                nc.vector.tensor_scalar(plzA[0:32, :], plx[0:32, 0:260],
                                        f0c32, None, ALU.mult)
                nc.vector.tensor_scalar(selt[0:32, :], plx[0:32, 256:516],
                                        f1c32, None, ALU.mult)
                nc.vector.tensor_add(plzA[0:32, :], plzA[0:32, :], selt[0:32, :])
                nc.sync.dma_start(out=plzA[32:33, :], in_=plmaskrow_s)

                # ---- V weight variants from flags ----
                nc.scalar.activation(VWf_t[0:32, :], VW_s, AF.Copy)
                nc.scalar.activation(VWq_t[0:32, :], VW_s, AF.Copy)
                nc.vector.tensor_scalar(Xf_t[0:32, :], X0P_s, f0c32, None,
                                        ALU.mult)
                for dr in range(3):
                    nc.vector.tensor_sub(
                        VWf_t[0:32, 128 * dr + 32:128 * dr + 64],
                        VWf_t[0:32, 128 * dr + 32:128 * dr + 64],
                        Xf_t[0:32, 32 * dr:32 * dr + 32])
                nc.vector.tensor_scalar(Xf_t[0:32, :], X2P_s, f1c32, None,
                                        ALU.mult)
                for dr in range(3):
                    nc.vector.tensor_sub(
                        VWq_t[0:32, 128 * dr + 64:128 * dr + 96],
                        VWq_t[0:32, 128 * dr + 64:128 * dr + 96],
                        Xf_t[0:32, 32 * dr:32 * dr + 32])

                # ---- U ----
                up = spsum.tile([128, 512], f32, tag="sp")
                for dl in range(3):
                    nc.tensor.matmul(up[0:32, 0:258],
                                     UW_s[0:32, 32 * dl:32 * dl + 32],
                                     plzA[0:32, dl:dl + 258],
                                     start=(dl == 0), stop=(dl == 2))
                nc.scalar.activation(U_sb[0:32, 0:258], up[0:32, 0:258], AF.Copy)

                # ---- c0 / c511 rows ----
                nc.sync.dma_start(out=A0AUG[0:32, :], in_=cpk32d[:, 96:192])
                nc.sync.dma_start(out=A511AUG[0:32, :], in_=cpk32d[:, 192:288])
                for which, (W1c_s, cc_s, dst) in enumerate(
                        ((W1c0_s, c0c_s, A0AUG), (W1c511_s, c511c_s, A511AUG))):
                    cp = spsum.tile([128, 512], f32, tag="sp")
                    for dl in range(3):
                        for t in range(2):
                            col = (1 + t) if which == 0 else (511 + t)
                            nc.tensor.matmul(
                                cp[0:1, 32 * dl:32 * dl + 32],
                                przA[0:32, col:col + 1],
                                W1c_s[0:32, 32 * (2 * dl + t):32 * (2 * dl + t) + 32],
                                start=(t == 0), stop=False)
                        nc.tensor.matmul(
                            cp[0:1, 32 * dl:32 * dl + 32],
                            ONE1_s,
                            cc_s[0:1, 32 * dl:32 * dl + 32],
                            start=False, stop=True)
                    nc.scalar.activation(dst[32:33, 0:96], cp[0:1, 0:96], AF.Copy)

                # ---- Ucol0 / Ucol511 ----
                for AUG, dstu in ((A0AUG, Uc0_sb), (A511AUG, Uc511_sb)):
                    ucp = spsum.tile([128, 512], f32, tag="sp")
                    for dl in range(3):
                        nc.tensor.matmul(ucp[0:32, 0:258],
                                         AUG[0:33, 32 * dl:32 * dl + 32],
                                         plzA[0:33, dl:dl + 258],
                                         start=(dl == 0), stop=(dl == 2))
                    nc.scalar.activation(dstu[0:32, 0:258], ucp[0:32, 0:258], AF.Copy)

                # ---- V variants ----
                for VWx, VCx, vt in ((VW_s, VC_s, V_rep),
                                     (VWf_t[0:32, :], VCf_s, V_first),
                                     (VWq_t[0:32, :], VCq_s, V_qlast)):
                    vp = spsum.tile([128, 512], f32, tag="sp")
                    for dr in range(3):
                        nc.tensor.matmul(vp[:, 0:512],
                                         VWx[0:32, 128 * dr:128 * dr + 128],
                                         przA[0:32, dr:dr + 512],
                                         start=(dr == 0), stop=False)
                    nc.tensor.matmul(vp[:, 0:512], VCx, ONESR_s,
                                     start=False, stop=True)
                    nc.scalar.activation(vt[:], vp[:, 0:512], AF.Copy)

                # ---- u relayouts (i = 4s+j for P, 4s+2+j for Q) ----
                for (src, dstP, dstQ) in ((U_sb, uP, uQ), (Uc0_sb, uc0P, uc0Q),
                                          (Uc511_sb, uc511P, uc511Q)):
                    srcr = src[0:32, 0:260].rearrange("c (s f) -> c s f", f=4)
                    for j in range(4):
                        nc.sync.dma_start(out=dstP[32 * j:32 * j + 32, 0:64],
                                          in_=srcr[:, 0:64, j])
                    for j in range(2):
                        nc.sync.dma_start(out=dstQ[32 * j:32 * j + 32, 0:64],
                                          in_=srcr[:, 0:64, 2 + j])
                    for j in range(2, 4):
                        nc.sync.dma_start(out=dstQ[32 * j:32 * j + 32, 0:64],
                                          in_=srcr[:, 1:65, j - 2])
                for (t, col, rm) in ((uP, 0, rmP0_s), (uc0P, 0, rmP0_s),
                                     (uc511P, 0, rmP0_s), (uQ, 63, rmQ63_s),
                                     (uc0Q, 63, rmQ63_s), (uc511Q, 63, rmQ63_s)):
                    nc.vector.tensor_mul(t[:, col:col + 1], t[:, col:col + 1], rm)

            # ================= conv pipeline =================
            with tc.tile_pool(name="x1", bufs=3) as x1pool, \
                 tc.tile_pool(name="x2", bufs=3) as x2pool, \
                 tc.tile_pool(name="osb", bufs=2) as opool, \
                 tc.tile_pool(name="cpsum", bufs=4, space="PSUM") as cpsum, \
                 tc.tile_pool(name="c3ps", bufs=2, space="PSUM") as c3psum:

                for k in range(NSTRIP):
                    x1P = x1pool.tile([128, 8 * 514], bf16, tag="x1P")
                    x1Q = x1pool.tile([128, 8 * 514], bf16, tag="x1Q")
                    for s in range(8):
                        sg = 8 * k + s
                        for (tile_, uu, Vgen, is_edge, rm) in (
                                (x1P, uP, V_first if sg == 0 else V_rep, sg == 0, rmP0_s),
                                (x1Q, uQ, V_qlast if sg == 63 else V_rep, sg == 63, rmQ63_s)):
                            dst = tile_[:, s * 514 + 1:s * 514 + 513]
                            bias_ap = uu[:, sg:sg + 1]
                            if is_edge:
                                nc.scalar.activation(dst, Vgen[:], AF.Relu,
                                                     bias=bias_ap, scale=rm)
                            elif s % 3 == 0:
                                nc.scalar.activation(dst, Vgen[:], AF.Relu, bias=bias_ap)
                            else:
                                nc.vector.tensor_scalar(dst, Vgen[:], bias_ap, 0.0,
                                                        ALU.add, ALU.max)
                    for tile_, ucol0, ucol511 in ((x1P, uc0P, uc511P), (x1Q, uc0Q, uc511Q)):
                        tr = tile_[:].rearrange("p (s c) -> p s c", c=514)
                        nc.vector.memset(tr[:, :, 0], 0.0)
                        nc.vector.memset(tr[:, :, 513], 0.0)
                        nc.vector.tensor_scalar(tr[:, :, 1], ucol0[:, 8 * k:8 * k + 8],
                                                0.0, None, ALU.max)
                        nc.vector.tensor_scalar(tr[:, :, 512], ucol511[:, 8 * k:8 * k + 8],
                                                0.0, None, ALU.max)

                    x2 = x2pool.tile([128, 8 * 512], bf16, tag="x2")
                    for s in range(8):
                        c2 = cpsum.tile([128, 512], f32, tag="c2")
                        for dr in range(3):
                            wp0 = W2P0_s[:, 32 * dr:32 * dr + 32]
                            wp1 = W2P1_s[:, 32 * dr:32 * dr + 32]
                            rhsP = x1P[:, s * 514 + dr:s * 514 + dr + 512]
                            rhsQ = x1Q[:, s * 514 + dr:s * 514 + dr + 512]
                            st, sp_ = (dr == 0), (dr == 2)
                            nc.tensor.matmul(c2[0:32, :], wp0, rhsP, start=st, stop=sp_,
                                             tile_position=(0, 0), skip_group_check=True)
                            nc.tensor.matmul(c2[32:64, :], wp1, rhsP, start=st, stop=sp_,
                                             tile_position=(0, 32), skip_group_check=True)
                            nc.tensor.matmul(c2[64:96, :], wp0, rhsQ, start=st, stop=sp_,
                                             tile_position=(0, 64), skip_group_check=True)
                            nc.tensor.matmul(c2[96:128, :], wp1, rhsQ, start=st, stop=sp_,
                                             tile_position=(0, 96), skip_group_check=True)
                        dst2 = x2[:, s * 512:(s + 1) * 512]
                        if s % 3 != 2:
                            nc.scalar.activation(dst2, c2[:], AF.Relu, bias=bc2rep_s)
                        else:
                            nc.vector.tensor_scalar(dst2, c2[:], bc2rep_s, 0.0,
                                                    ALU.add, ALU.max)

                    # conv3: logits transposed onto 128 partitions (r-slab on
                    # partitions, strip-row on free); undone host-side.
                    c3p = c3psum.tile([128, 128], f32, tag="c3")
                    for s in range(8):
                        xc = x2[:, s * 512:(s + 1) * 512]
                        for u in range(4):
                            nc.tensor.matmul(
                                c3p[:, 32 * u + 4 * s:32 * u + 4 * s + 4],
                                xc[:, 128 * u:128 * u + 128],
                                W3sel_s, start=True, stop=True)
                    sgt = opool.tile([128, 128], f32, tag="sgt")
                    nc.scalar.activation(sgt[:], c3p[:], AF.Sigmoid,
                                         bias=b3vec_s)
                    osb = opool.tile([128, 128], u8, tag="osb")
                    nc.vector.tensor_scalar(osb[:], sgt[:], 255.0, None, ALU.mult)
                    # osb[p, 32u+4s+m] = 255*sigmoid(logit[row=4s+m, r=128u+p])
                    osr = osb[:].rearrange("p (u c) -> p u c", c=32)
                    outr = out[:].rearrange("(u p) g -> p u g", p=128)
                    nc.sync.dma_start(out=outr[:, :, 32 * k:32 * k + 32],
                                      in_=osr)

    nc.compile()
    return nc


def kernel(**inputs):
    from concourse.bass_utils import run_bass_kernel_spmd
    if "nc" not in _CACHE:
        _CACHE["nc"] = _build_program()
    nc = _CACHE["nc"]
    maps = _host_prep(inputs)
    res = run_bass_kernel_spmd(nc, maps, core_ids=list(range(8)))
    _CACHE["last_result"] = res
    full = np.zeros((B, L, R), np.float32)
    for core in range(8):
        b, h = core // 2, core % 2
        full[b, 256 * h:256 * h + 256, :] = \
            (res.results[core]["out"].astype(np.float32) / 255.0).T
    return full


# revision 4
# speedup vs baseline: 1.1892x; 1.1892x over previous
"""ProteinInterfacePrediction fused Bass kernel for 8 TRN2 NeuronCores.

Sharding: core c = (batch b = c//2, half h = c%2); each core computes the
(256, 512) output tile for L-rows [256h, 256h+256).

GNN dedupe: within a batch pair, the EVEN core runs the receptor GNN and the
ODD core runs the ligand GNN (full 512 nodes each); the (32,512) HOPI
projections are exchanged on-chip via a pairwise AllGather, so every edge
byte is shipped to the device exactly once.

Decomposition (validated bit-level in numpy vs the jax reference):
  - GNN residual folded into HOPI: proj = Wp@nodeT + (Wp/16)@S, S = sum_k tanh(hn+he)
  - conv1 is rank-separable before relu: conv1(P) = U[co,l] + V[co,r] (+consts),
    boundary columns via mask-augmented 1-D convs, boundary rows via flag-built
    V-weight variants.
  - conv2 on TensorE: 4-input-row blocks on 128 partitions (K = 4rows x 32ci),
    stride-2 (P/Q dual layouts), 3 dr-taps, 4-way 32-column array tiling.
  - conv3 (1x1) + bias + sigmoid fused at the tail.

Wire-format optimizations (the harness metric is wall-clock of
run_bass_kernel_spmd, dominated by host<->device transfer + dispatch):
  - edge features shipped as fp8e4m3, nodes + weights as bf16 (adds ~4e-4
    rel err vs the 2e-2 budget)
  - all small constants packed into 4 tensors (8 inputs/core)
  - output shipped as uint8 (sigmoid * 255; quantization err <= 1/255)
  - persistent jax compilation cache so the per-call jit rebuild inside
    run_bass_via_pjrt hits disk instead of recompiling XLA
"""

import numpy as np
import ml_dtypes

try:  # make the per-call jit re-lowering inside run_bass_via_pjrt cacheable
    import jax as _jax
    _jax.config.update("jax_compilation_cache_dir", "/tmp/jaxcache")
    _jax.config.update("jax_persistent_cache_min_compile_time_secs", 0.0)
    _jax.config.update("jax_persistent_cache_min_entry_size_bytes", -1)
except Exception:
    pass

B, L, R, KNB = 4, 512, 512, 16
DN, DE = 128, 64
NN = 512                 # nodes per GNN (one full molecule per core)
PN = NN * KNB
CH = 64                  # gnn nodes per chunk
NSTRIP = 8

_CACHE = {}

# packed-constant column layouts
_BPK = dict(WNT=(0, 128), WpT=(128, 160), W2cat=(160, 352), W3selb=(352, 356))
_CPK128 = dict(gnnbias=(0, 1), WpT16=(1, 33), bc2rep=(33, 34), b3vec=(34, 35),
               rmP0=(35, 36), rmQ63=(36, 37), f0col=(37, 38), f1col=(38, 39))
_CPK32 = dict(UW=(0, 96), A0W=(96, 192), A511W=(192, 288), W1c0=(288, 480),
              W1c511=(480, 672), VW=(672, 1056), X0P=(1056, 1152),
              X2P=(1152, 1248))
_CPK1 = dict(c0const=(0, 96), c511const=(96, 192), VC=(192, 320),
             VCfirst=(320, 448), VCqlast=(448, 576), ONE1=(576, 577),
             ONESR=(577, 1089), plmaskrow=(1089, 1349))


def _host_prep(inputs):
    f32 = np.float32
    bf16 = ml_dtypes.bfloat16
    fp8 = ml_dtypes.float8_e4m3  # == mybir.dt.np(dt.float8e4)
    W1 = np.asarray(inputs['Wc1'], f32)
    W2 = np.asarray(inputs['Wc2'], f32)
    W3 = np.asarray(inputs['Wc3'], f32)[0, :, 0, 0]
    b1 = np.asarray(inputs['bc1'], f32)
    b2 = np.asarray(inputs['bc2'], f32)
    b3 = float(np.asarray(inputs['bc3'], f32)[0])
    Wp = np.asarray(inputs['Wp'], f32)
    bp = np.asarray(inputs['bp'], f32)
    Wl, Wr = Wp[:, :DN], Wp[:, DN:]
    WN = np.asarray(inputs['WN'], f32)
    bN = np.asarray(inputs['bN'], f32)
    WE = np.asarray(inputs['WE'], f32)
    bE = np.asarray(inputs['bE'], f32)

    A = W1.sum(axis=3)
    Wv = W1.sum(axis=2)
    cU = np.einsum('oidr,i->od', W1, bp)

    # ---- shared bf16 pack pieces (WpT slot filled per-core) ----
    bpk0 = np.zeros((128, 356), bf16)

    def bput(pk, name, arr):
        a, b_ = _BPK[name]
        pk[:arr.shape[0], a:b_] = arr.astype(bf16)

    bput(bpk0, 'WNT', np.ascontiguousarray(WN.T))
    W2P0 = np.zeros((128, 96), f32)
    W2P1 = np.zeros((128, 96), f32)
    for dr in range(3):
        for j in range(3):
            W2P0[32 * j:32 * j + 32, 32 * dr:32 * dr + 32] = W2[:, :, j, dr].T
        for j in range(1, 4):
            W2P1[32 * j:32 * j + 32, 32 * dr:32 * dr + 32] = W2[:, :, j - 1, dr].T
    W2cat = np.zeros((128, 192), f32)
    for dr in range(3):
        W2cat[:, 64 * dr:64 * dr + 32] = W2P0[:, 32 * dr:32 * dr + 32]
        W2cat[:, 64 * dr + 32:64 * dr + 64] = W2P1[:, 32 * dr:32 * dr + 32]
    bput(bpk0, 'W2cat', W2cat)
    W3sel = np.zeros((128, 4), f32)
    for j in range(4):
        W3sel[32 * j:32 * j + 32, j] = W3
    bput(bpk0, 'W3selb', W3sel)

    # ---- shared 32-row f32 pieces ----
    def pack3(M):  # (co, ci, dl) -> [32, 96] of [ci, co] blocks
        out = np.zeros((32, 96), f32)
        for dl in range(3):
            out[:, 32 * dl:32 * dl + 32] = M[:, :, dl].T
        return out

    cpk32_0 = np.zeros((32, 1248), f32)

    def c32put(name, arr):
        a, b_ = _CPK32[name]
        cpk32_0[:arr.shape[0], a:b_] = arr

    c32put('UW', pack3(A))
    c32put('A0W', pack3(W1[:, :, :, 1:].sum(axis=3)))
    c32put('A511W', pack3(W1[:, :, :, :2].sum(axis=3)))

    W1c0 = np.zeros((32, 192), f32)
    W1c511 = np.zeros((32, 192), f32)
    for dl in range(3):
        for t, dr in enumerate((1, 2)):
            W1c0[:, 32 * (2 * dl + t):32 * (2 * dl + t) + 32] = W1[:, :, dl, dr].T
        for t, dr in enumerate((0, 1)):
            W1c511[:, 32 * (2 * dl + t):32 * (2 * dl + t) + 32] = W1[:, :, dl, dr].T
    c32put('W1c0', W1c0)
    c32put('W1c511', W1c511)

    VW = np.zeros((32, 384), f32)
    for dr in range(3):
        blk = Wv[:, :, dr].T
        for j in range(4):
            VW[:, 128 * dr + 32 * j:128 * dr + 32 * j + 32] = blk
    c32put('VW', VW)
    X0P = np.zeros((32, 96), f32)
    X2P = np.zeros((32, 96), f32)
    for dr in range(3):
        X0P[:, 32 * dr:32 * dr + 32] = W1[:, :, 0, dr].T
        X2P[:, 32 * dr:32 * dr + 32] = W1[:, :, 2, dr].T
    c32put('X0P', X0P)
    c32put('X2P', X2P)

    # ---- shared 1-row f32 pieces (VCfirst/VCqlast flag-baked per core) ----
    c0c = np.zeros((1, 96), f32)
    c511c = np.zeros((1, 96), f32)
    for dl in range(3):
        c0c[0, 32 * dl:32 * dl + 32] = np.einsum('oid,i->o', W1[:, :, dl, 1:], bp)
        c511c[0, 32 * dl:32 * dl + 32] = np.einsum('oid,i->o', W1[:, :, dl, :2], bp)
    c0c[0, 32:64] += b1
    c511c[0, 32:64] += b1
    vc = cU.sum(axis=1) + b1
    VC = np.tile(vc, 4).reshape(1, 128).astype(f32)

    sh = {'wfp8': np.ascontiguousarray(WE.T).astype(fp8)}

    lig_nf = np.asarray(inputs['ligand_node_features'], f32)
    lig_ef = np.asarray(inputs['ligand_edge_features'], f32)
    rec_nf = np.asarray(inputs['receptor_node_features'], f32)
    rec_ef = np.asarray(inputs['receptor_edge_features'], f32)

    maps = []
    for core in range(8):
        b, h = core // 2, core % 2
        lo = 256 * h - 2
        m = dict(sh)

        # even core: receptor GNN; odd core: ligand GNN
        if h == 0:
            nf, ef, Wpp = rec_nf[b], rec_ef[b], Wr
        else:
            nf, ef, Wpp = lig_nf[b], lig_ef[b], Wl
        m['nodeT'] = np.ascontiguousarray(nf.T).astype(fp8)
        m['edgeT'] = np.ascontiguousarray(ef.reshape(PN, DE).T).astype(fp8)

        bpk = bpk0.copy()
        bput(bpk, 'WpT', np.ascontiguousarray(Wpp.T))
        m['bpk'] = bpk

        cpk128 = np.zeros((128, 39), f32)

        def c128put(name, arr):
            a, b_ = _CPK128[name]
            cpk128[:arr.shape[0], a:b_] = arr

        c128put('gnnbias', (bN + bE).reshape(DN, 1))
        c128put('WpT16', np.ascontiguousarray((Wpp / 16.0).T)[:, 0:32])
        c128put('bc2rep', np.tile(b2, 4).reshape(128, 1))
        c128put('b3vec', np.full((128, 1), b3, f32))
        flag0 = 1.0 if h == 0 else 0.0
        flag1 = 1.0 if h == 1 else 0.0
        c128put('f0col', np.full((128, 1), flag0, f32))
        c128put('f1col', np.full((128, 1), flag1, f32))
        rmP0 = np.ones((128, 1), f32)
        rmQ63 = np.ones((128, 1), f32)
        for j in range(4):
            if not (0 <= 256 * h + (j - 1) < L):
                rmP0[32 * j:32 * j + 32] = 0.0
            if not (0 <= 256 * h + (253 + j) < L):
                rmQ63[32 * j:32 * j + 32] = 0.0
        c128put('rmP0', rmP0)
        c128put('rmQ63', rmQ63)
        m['cpk128'] = cpk128

        # cpk32 rides the pair AllGather: even core ships rows 0:16, odd 16:32
        m['cpk32h'] = np.ascontiguousarray(cpk32_0[16 * h:16 * h + 16])

        cpk1 = np.zeros((1, 1349), f32)

        def c1put(name, arr):
            a, b_ = _CPK1[name]
            cpk1[:, a:b_] = arr

        c1put('c0const', c0c)
        c1put('c511const', c511c)
        VCfirst, VCqlast = VC.copy(), VC.copy()
        VCfirst[0, 32:64] -= flag0 * cU[:, 0]
        VCqlast[0, 64:96] -= flag1 * cU[:, 2]
        c1put('VC', VC)
        c1put('VCfirst', VCfirst)
        c1put('VCqlast', VCqlast)
        c1put('ONE1', np.ones((1, 1), f32))
        c1put('ONESR', np.ones((1, 512), f32))
        plmask = np.array([1.0 if 0 <= lo + i < L else 0.0 for i in range(260)],
                          f32)
        c1put('plmaskrow', plmask.reshape(1, 260))
        m['cpk1'] = cpk1
        maps.append(m)
    return maps


def _build_program():
    import concourse.bacc as bacc
    import concourse.mybir as mybir
    from concourse.tile import TileContext

    dt = mybir.dt
    f32, bf16, fp8, u8 = dt.float32, dt.bfloat16, dt.float8e4, dt.uint8
    AF = mybir.ActivationFunctionType
    ALU = mybir.AluOpType

    nc = bacc.Bacc("TRN2", target_bir_lowering=False, debug=False, num_devices=8)

    def din(name, shape, dtype=f32):
        return nc.dram_tensor(name, list(shape), dtype, kind="ExternalInput")

    nodeTd = din("nodeT", (128, NN), fp8)
    edgeTd = din("edgeT", (64, PN), fp8)
    wfp8d = din("wfp8", (64, 128), fp8)
    bpkd = din("bpk", (128, 356), bf16)
    cpk128d = din("cpk128", (128, 39))
    cpk32hd = din("cpk32h", (16, 1248))
    cpk1d = din("cpk1", (1, 1349))
    out = nc.dram_tensor("out", [512, 256], u8, kind="ExternalOutput")

    with TileContext(nc) as tc:
        with tc.tile_pool(name="const", bufs=1) as cpool, \
             tc.tile_pool(name="dram", bufs=1, space="DRAM") as dpool:
            WETb_s = cpool.tile([128, 128], fp8, tag="wfp8")
            nc.sync.dma_start(out=WETb_s[0:64, :], in_=wfp8d[:])
            bpk_s = cpool.tile([128, 356], bf16, tag="bpk")
            nc.sync.dma_start(out=bpk_s[:], in_=bpkd[:])
            cpk128_s = cpool.tile([128, 39], f32, tag="cpk128")
            nc.sync.dma_start(out=cpk128_s[:], in_=cpk128d[:])
            cpk32_s = cpool.tile([128, 1248], f32, tag="cpk32")
            cpk1_s = cpool.tile([128, 1349], f32, tag="cpk1")
            nc.sync.dma_start(out=cpk1_s[0:1, :], in_=cpk1d[:])
            nodeTb_s = cpool.tile([128, NN], bf16, tag="nodeTb")
            nc.gpsimd.dma_start(out=nodeTb_s[:], in_=nodeTd[:])

            def bsl(name, rows=128):
                a, b_ = _BPK[name]
                return bpk_s[0:rows, a:b_]

            def c128sl(name, rows=128):
                a, b_ = _CPK128[name]
                return cpk128_s[0:rows, a:b_]

            def c32sl(name, rows=32):
                a, b_ = _CPK32[name]
                return cpk32_s[0:rows, a:b_]

            def c1sl(name):
                a, b_ = _CPK1[name]
                return cpk1_s[0:1, a:b_]

            WNT_s = bsl('WNT')
            WpT_s = bsl('WpT')
            W2cat_s = bsl('W2cat')
            W3sel_s = bsl('W3selb')
            gnnbias_s = c128sl('gnnbias')
            WpT16_s = c128sl('WpT16')
            bc2rep_s, b3vec_s = c128sl('bc2rep'), c128sl('b3vec')
            rmP0_s, rmQ63_s = c128sl('rmP0'), c128sl('rmQ63')
            f0col_s, f1col_s = c128sl('f0col'), c128sl('f1col')
            UW_s = c32sl('UW')
            W1c0_s, W1c511_s = c32sl('W1c0'), c32sl('W1c511')
            VW_s = c32sl('VW')
            X0P_s, X2P_s = c32sl('X0P'), c32sl('X2P')
            c0c_s, c511c_s = c1sl('c0const'), c1sl('c511const')
            VC_s, VCf_s, VCq_s = c1sl('VC'), c1sl('VCfirst'), c1sl('VCqlast')
            ONE1_s, ONESR_s = c1sl('ONE1'), c1sl('ONESR')
            plmaskrow_s = c1sl('plmaskrow')

            S_t = cpool.tile([128, NN], f32)
            projm = cpool.tile([128, 512], f32)   # rows 0:32: own projection
            plx = cpool.tile([128, 516], f32)     # rows 0:32: [0,0, pl, 0,0]
            selt = cpool.tile([128, 260], f32)
            plzA = cpool.tile([128, 260], f32)    # rows 0-31 plz, row 32 mask
            przA = cpool.tile([128, 514], f32)
            U_sb = cpool.tile([128, 260], f32)
            Uc0_sb = cpool.tile([128, 260], f32)
            Uc511_sb = cpool.tile([128, 260], f32)
            A0AUG = cpool.tile([128, 96], f32)
            A511AUG = cpool.tile([128, 96], f32)
            VWf_t = cpool.tile([128, 384], f32)
            VWq_t = cpool.tile([128, 384], f32)
            Xf_t = cpool.tile([128, 96], f32)
            V_rep = cpool.tile([128, 512], f32)
            V_first = cpool.tile([128, 512], f32)
            V_qlast = cpool.tile([128, 512], f32)
            uP = cpool.tile([128, 64], f32, tag="uP")
            uQ = cpool.tile([128, 64], f32, tag="uQ")
            uc0P = cpool.tile([128, 64], f32, tag="uc0P")
            uc0Q = cpool.tile([128, 64], f32, tag="uc0Q")
            uc511P = cpool.tile([128, 64], f32, tag="uc511P")
            uc511Q = cpool.tile([128, 64], f32, tag="uc511Q")

            # ================= GNN phase (own molecule only) =================
            with tc.tile_pool(name="gnn", bufs=4) as gpool, \
                 tc.tile_pool(name="gpsum", bufs=3, space="PSUM") as gpsum, \
                 tc.tile_pool(name="spsum", bufs=1, space="PSUM") as spsum:

                for c in range(PN // (CH * KNB)):
                    et = gpool.tile([128, CH * KNB], fp8, tag="edge")
                    nc.gpsimd.dma_start(
                        out=et[0:64, :],
                        in_=edgeTd[:, c * CH * KNB:(c + 1) * CH * KNB])
                    hz = gpsum.tile([128, CH * KNB], f32, tag="hz")
                    for q in range(CH * KNB // 512):
                        nc.tensor.matmul(
                            hz[:, q * 512:(q + 1) * 512],
                            WETb_s[0:64, :],
                            et[0:64, q * 512:(q + 1) * 512],
                            start=True, stop=False)
                        rhs = nodeTb_s[:, c * CH + q * 32:c * CH + (q + 1) * 32]
                        rhs = rhs.unsqueeze(2).broadcast_to([128, 32, KNB])
                        nc.tensor.matmul(
                            hz[:, q * 512:(q + 1) * 512],
                            WNT_s, rhs,
                            start=False, stop=True)
                    zt = gpool.tile([128, CH * KNB], bf16, tag="zt")
                    nc.scalar.activation(zt[:], hz[:], AF.Tanh, bias=gnnbias_s)
                    ztr = zt[:].rearrange("p (n k) -> p n k", k=KNB)
                    nc.vector.reduce_sum(
                        S_t[:, c * CH:(c + 1) * CH], ztr,
                        axis=mybir.AxisListType.X)

                # ---- HOPI projection of own molecule ----
                pp = spsum.tile([128, 512], f32, tag="sp")
                nc.tensor.matmul(pp[0:32, 0:NN], WpT_s, nodeTb_s[:],
                                 start=True, stop=False)
                nc.tensor.matmul(pp[0:32, 0:NN], WpT16_s, S_t[:],
                                 start=False, stop=True)
                nc.scalar.activation(projm[0:32, :], pp[0:32, 0:NN], AF.Copy)

                # ---- pairwise exchange: cols 0:1024 carry the (32,512) proj
                # viewed as (16,1024); cols 1024:2272 carry this core's half of
                # cpk32. Gathered rows 0:16 = even core (pr + cpk32[0:16]),
                # rows 16:32 = odd core (pl + cpk32[16:32]). ----
                cin = dpool.tile([16, 2272], f32, tag="cin")
                cout = dpool.tile([32, 2272], f32, tag="cout")
                nc.gpsimd.dma_start(out=cin[:, 0:512], in_=projm[0:16, :])
                nc.gpsimd.dma_start(out=cin[:, 512:1024], in_=projm[16:32, :])
                nc.gpsimd.dma_start(out=cin[:, 1024:2272], in_=cpk32hd[:])
                nc.gpsimd.collective_compute(
                    "AllGather", ALU.bypass,
                    replica_groups=[[0, 1], [2, 3], [4, 5], [6, 7]],
                    ins=[cin.opt()], outs=[cout.opt()])
                nc.vector.memset(przA[0:32, 0:1], 0.0)
                nc.vector.memset(przA[0:32, 513:514], 0.0)
                nc.sync.dma_start(out=przA[0:16, 1:513], in_=cout[0:16, 0:512])
                nc.sync.dma_start(out=przA[16:32, 1:513], in_=cout[0:16, 512:1024])
                nc.vector.memset(plx[0:32, 0:2], 0.0)
                nc.vector.memset(plx[0:32, 514:516], 0.0)
                nc.sync.dma_start(out=plx[0:16, 2:514], in_=cout[16:32, 0:512])
                nc.sync.dma_start(out=plx[16:32, 2:514], in_=cout[16:32, 512:1024])
                nc.sync.dma_start(out=cpk32_s[0:32, :], in_=cout[:, 1024:2272])

                # ---- plzA = f0*plx[:,0:260] + f1*plx[:,256:516]; row 32 mask ----
                f0c32, f1c32 = c128sl('f0col', rows=32), c128sl('f1col', rows=32)
                nc.vector.tensor_scalar(plzA[0:32, :], plx[0:32, 0:260],
                                        f0c32, None, ALU.mult)
                nc.vector.tensor_scalar(selt[0:32, :], plx[0:32, 256:516],
                                        f1c32, None, ALU.mult)
                nc.vector.tensor_add(plzA[0:32, :], plzA[0:32, :], selt[0:32, :])
                nc.sync.dma_start(out=plzA[32:33, :], in_=plmaskrow_s)

                # ---- V weight variants from flags ----
                nc.scalar.activation(VWf_t[0:32, :], VW_s, AF.Copy)
                nc.scalar.activation(VWq_t[0:32, :], VW_s, AF.Copy)
                nc.vector.tensor_scalar(Xf_t[0:32, :], X0P_s, f0c32, None,
                                        ALU.mult)
                for dr in range(3):
                    nc.vector.tensor_sub(
                        VWf_t[0:32, 128 * dr + 32:128 * dr + 64],
                        VWf_t[0:32, 128 * dr + 32:128 * dr + 64],
                        Xf_t[0:32, 32 * dr:32 * dr + 32])
                nc.vector.tensor_scalar(Xf_t[0:32, :], X2P_s, f1c32, None,
                                        ALU.mult)
                for dr in range(3):
                    nc.vector.tensor_sub(
                        VWq_t[0:32, 128 * dr + 64:128 * dr + 96],
                        VWq_t[0:32, 128 * dr + 64:128 * dr + 96],
                        Xf_t[0:32, 32 * dr:32 * dr + 32])

                # ---- U ----
                up = spsum.tile([128, 512], f32, tag="sp")
                for dl in range(3):
                    nc.tensor.matmul(up[0:32, 0:258],
                                     UW_s[0:32, 32 * dl:32 * dl + 32],
                                     plzA[0:32, dl:dl + 258],
                                     start=(dl == 0), stop=(dl == 2))
                nc.scalar.activation(U_sb[0:32, 0:258], up[0:32, 0:258], AF.Copy)

                # ---- c0 / c511 rows ----
                nc.sync.dma_start(out=A0AUG[0:32, :],
                                  in_=cout[:, 1024 + 96:1024 + 192])
                nc.sync.dma_start(out=A511AUG[0:32, :],
                                  in_=cout[:, 1024 + 192:1024 + 288])
                for which, (W1c_s, cc_s, dst) in enumerate(
                        ((W1c0_s, c0c_s, A0AUG), (W1c511_s, c511c_s, A511AUG))):
                    cp = spsum.tile([128, 512], f32, tag="sp")
                    for dl in range(3):
                        for t in range(2):
                            col = (1 + t) if which == 0 else (511 + t)
                            nc.tensor.matmul(
                                cp[0:1, 32 * dl:32 * dl + 32],
                                przA[0:32, col:col + 1],
                                W1c_s[0:32, 32 * (2 * dl + t):32 * (2 * dl + t) + 32],
                                start=(t == 0), stop=False)
                        nc.tensor.matmul(
                            cp[0:1, 32 * dl:32 * dl + 32],
                            ONE1_s,
                            cc_s[0:1, 32 * dl:32 * dl + 32],
                            start=False, stop=True)
                    nc.scalar.activation(dst[32:33, 0:96], cp[0:1, 0:96], AF.Copy)

                # ---- Ucol0 / Ucol511 ----
                for AUG, dstu in ((A0AUG, Uc0_sb), (A511AUG, Uc511_sb)):
                    ucp = spsum.tile([128, 512], f32, tag="sp")
                    for dl in range(3):
                        nc.tensor.matmul(ucp[0:32, 0:258],
                                         AUG[0:33, 32 * dl:32 * dl + 32],
                                         plzA[0:33, dl:dl + 258],
                                         start=(dl == 0), stop=(dl == 2))
                    nc.scalar.activation(dstu[0:32, 0:258], ucp[0:32, 0:258], AF.Copy)

                # ---- V variants ----
                for VWx, VCx, vt in ((VW_s, VC_s, V_rep),
                                     (VWf_t[0:32, :], VCf_s, V_first),
                                     (VWq_t[0:32, :], VCq_s, V_qlast)):
                    vp = spsum.tile([128, 512], f32, tag="sp")
                    for dr in range(3):
                        nc.tensor.matmul(vp[:, 0:512],
                                         VWx[0:32, 128 * dr:128 * dr + 128],
                                         przA[0:32, dr:dr + 512],
                                         start=(dr == 0), stop=False)
                    nc.tensor.matmul(vp[:, 0:512], VCx, ONESR_s,
                                     start=False, stop=True)
                    nc.scalar.activation(vt[:], vp[:, 0:512], AF.Copy)

                # ---- u relayouts (i = 4s+j for P, 4s+2+j for Q) ----
                for (src, dstP, dstQ) in ((U_sb, uP, uQ), (Uc0_sb, uc0P, uc0Q),
                                          (Uc511_sb, uc511P, uc511Q)):
                    srcr = src[0:32, 0:260].rearrange("c (s f) -> c s f", f=4)
                    for j in range(4):
                        nc.sync.dma_start(out=dstP[32 * j:32 * j + 32, 0:64],
                                          in_=srcr[:, 0:64, j])
                    for j in range(2):
                        nc.sync.dma_start(out=dstQ[32 * j:32 * j + 32, 0:64],
                                          in_=srcr[:, 0:64, 2 + j])
                    for j in range(2, 4):
                        nc.sync.dma_start(out=dstQ[32 * j:32 * j + 32, 0:64],
                                          in_=srcr[:, 1:65, j - 2])
                for (t, col, rm) in ((uP, 0, rmP0_s), (uc0P, 0, rmP0_s),
                                     (uc511P, 0, rmP0_s), (uQ, 63, rmQ63_s),
                                     (uc0Q, 63, rmQ63_s), (uc511Q, 63, rmQ63_s)):
                    nc.vector.tensor_mul(t[:, col:col + 1], t[:, col:col + 1], rm)

            # ================= conv pipeline =================
            with tc.tile_pool(name="x1", bufs=3) as x1pool, \
                 tc.tile_pool(name="x2", bufs=3) as x2pool, \
                 tc.tile_pool(name="osb", bufs=2) as opool, \
                 tc.tile_pool(name="cpsum", bufs=4, space="PSUM") as cpsum, \
                 tc.tile_pool(name="c3ps", bufs=2, space="PSUM") as c3psum:

                for k in range(NSTRIP):
                    x1P = x1pool.tile([128, 8 * 514], bf16, tag="x1P")
                    x1Q = x1pool.tile([128, 8 * 514], bf16, tag="x1Q")
                    for s in range(8):
                        sg = 8 * k + s
                        for (tile_, uu, Vgen, is_edge, rm) in (
                                (x1P, uP, V_first if sg == 0 else V_rep, sg == 0, rmP0_s),
                                (x1Q, uQ, V_qlast if sg == 63 else V_rep, sg == 63, rmQ63_s)):
                            dst = tile_[:, s * 514 + 1:s * 514 + 513]
                            bias_ap = uu[:, sg:sg + 1]
                            if is_edge:
                                nc.scalar.activation(dst, Vgen[:], AF.Relu,
                                                     bias=bias_ap, scale=rm)
                            elif s % 3 == 0:
                                nc.scalar.activation(dst, Vgen[:], AF.Relu, bias=bias_ap)
                            else:
                                nc.vector.tensor_scalar(dst, Vgen[:], bias_ap, 0.0,
                                                        ALU.add, ALU.max)
                    for tile_, ucol0, ucol511 in ((x1P, uc0P, uc511P), (x1Q, uc0Q, uc511Q)):
                        tr = tile_[:].rearrange("p (s c) -> p s c", c=514)
                        nc.vector.memset(tr[:, :, 0], 0.0)
                        nc.vector.memset(tr[:, :, 513], 0.0)
                        nc.vector.tensor_scalar(tr[:, :, 1], ucol0[:, 8 * k:8 * k + 8],
                                                0.0, None, ALU.max)
                        nc.vector.tensor_scalar(tr[:, :, 512], ucol511[:, 8 * k:8 * k + 8],
                                                0.0, None, ALU.max)

                    x2 = x2pool.tile([128, 8 * 512], bf16, tag="x2")
                    for s in range(8):
                        c2 = cpsum.tile([128, 512], f32, tag="c2")
                        for dr in range(3):
                            wcat = W2cat_s[:, 64 * dr:64 * dr + 64]
                            rhsP = x1P[:, s * 514 + dr:s * 514 + dr + 512]
                            rhsQ = x1Q[:, s * 514 + dr:s * 514 + dr + 512]
                            st, sp_ = (dr == 0), (dr == 2)
                            nc.tensor.matmul(c2[0:64, :], wcat, rhsP, start=st, stop=sp_,
                                             tile_position=(0, 0), skip_group_check=True)
                            nc.tensor.matmul(c2[64:128, :], wcat, rhsQ, start=st, stop=sp_,
                                             tile_position=(0, 64), skip_group_check=True)
                        dst2 = x2[:, s * 512:(s + 1) * 512]
                        if s % 3 != 2:
                            nc.scalar.activation(dst2, c2[:], AF.Relu, bias=bc2rep_s)
                        else:
                            nc.vector.tensor_scalar(dst2, c2[:], bc2rep_s, 0.0,
                                                    ALU.add, ALU.max)

                    # conv3: logits transposed onto 128 partitions (r-slab on
                    # partitions, strip-row on free); undone host-side.
                    c3p = c3psum.tile([128, 128], f32, tag="c3")
                    for s in range(8):
                        xc = x2[:, s * 512:(s + 1) * 512]
                        for u in range(4):
                            nc.tensor.matmul(
                                c3p[:, 32 * u + 4 * s:32 * u + 4 * s + 4],
                                xc[:, 128 * u:128 * u + 128],
                                W3sel_s, start=True, stop=True)
                    sgt = opool.tile([128, 128], f32, tag="sgt")
                    nc.scalar.activation(sgt[:], c3p[:], AF.Sigmoid,
                                         bias=b3vec_s)
                    osb = opool.tile([128, 128], u8, tag="osb")
                    nc.vector.tensor_scalar(osb[:], sgt[:], 255.0, None, ALU.mult)
                    # osb[p, 32u+4s+m] = 255*sigmoid(logit[row=4s+m, r=128u+p])
                    osr = osb[:].rearrange("p (u c) -> p u c", c=32)
                    outr = out[:].rearrange("(u p) g -> p u g", p=128)
                    nc.sync.dma_start(out=outr[:, :, 32 * k:32 * k + 32],
                                      in_=osr)

    nc.compile()
    return nc


def kernel(**inputs):
    from concourse.bass_utils import run_bass_kernel_spmd
    if "nc" not in _CACHE:
        _CACHE["nc"] = _build_program()
    nc = _CACHE["nc"]
    maps = _host_prep(inputs)
    res = run_bass_kernel_spmd(nc, maps, core_ids=list(range(8)))
    _CACHE["last_result"] = res
    full = np.zeros((B, L, R), np.float32)
    for core in range(8):
        b, h = core // 2, core % 2
        full[b, 256 * h:256 * h + 256, :] = \
            (res.results[core]["out"].astype(np.float32) / 255.0).T
    return full


# revision 5
# speedup vs baseline: 1.2383x; 1.0412x over previous
"""ProteinInterfacePrediction fused Bass kernel for 8 TRN2 NeuronCores.

Sharding: core c = (batch b = c//2, half h = c%2); each core computes the
(256, 512) output tile for L-rows [256h, 256h+256).

GNN dedupe: within a batch pair, the EVEN core runs the receptor GNN and the
ODD core runs the ligand GNN (full 512 nodes each); the (32,512) HOPI
projections are exchanged on-chip via a pairwise AllGather, so every edge
byte is shipped to the device exactly once.

Decomposition (validated bit-level in numpy vs the jax reference):
  - GNN residual folded into HOPI: proj = Wp@nodeT + (Wp/16)@S, S = sum_k tanh(hn+he)
  - conv1 is rank-separable before relu: conv1(P) = U[co,l] + V[co,r] (+consts),
    boundary columns via mask-augmented 1-D convs, boundary rows via flag-built
    V-weight variants.
  - conv2 on TensorE: 4-input-row blocks on 128 partitions (K = 4rows x 32ci),
    stride-2 (P/Q dual layouts), 3 dr-taps, 4-way 32-column array tiling.
  - conv3 (1x1) + bias + sigmoid fused at the tail.

Wire-format optimizations (the harness metric is wall-clock of
run_bass_kernel_spmd, dominated by host<->device transfer + dispatch):
  - edge features shipped as fp8e4m3, nodes + weights as bf16 (adds ~4e-4
    rel err vs the 2e-2 budget)
  - all small constants packed into 4 tensors (8 inputs/core)
  - output shipped as uint8 (sigmoid * 255; quantization err <= 1/255)
  - persistent jax compilation cache so the per-call jit rebuild inside
    run_bass_via_pjrt hits disk instead of recompiling XLA
"""

import numpy as np
import ml_dtypes

try:  # make the per-call jit re-lowering inside run_bass_via_pjrt cacheable
    import jax as _jax
    _jax.config.update("jax_compilation_cache_dir", "/tmp/jaxcache")
    _jax.config.update("jax_persistent_cache_min_compile_time_secs", 0.0)
    _jax.config.update("jax_persistent_cache_min_entry_size_bytes", -1)
except Exception:
    pass

B, L, R, KNB = 4, 512, 512, 16
DN, DE = 128, 64
NN = 512                 # nodes per GNN (one full molecule per core)
PN = NN * KNB
CH = 64                  # gnn nodes per chunk
NSTRIP = 8

_CACHE = {}

# packed-constant column layouts
_BPK = dict(WNT=(0, 128), WpT=(128, 160), W2cat=(160, 352), W3selb=(352, 356))
_CPK128 = dict(gnnbias=(0, 1), WpT16=(1, 33), bc2rep=(33, 34), b3vec=(34, 35),
               rmP0=(35, 36), rmQ63=(36, 37), f0col=(37, 38), f1col=(38, 39))
_CPK32 = dict(UW=(0, 96), A0W=(96, 192), A511W=(192, 288), W1c0=(288, 480),
              W1c511=(480, 672), VW=(672, 1056), X0P=(1056, 1152),
              X2P=(1152, 1248))
_CPK1 = dict(c0const=(0, 96), c511const=(96, 192), VC=(192, 320),
             VCfirst=(320, 448), VCqlast=(448, 576), ONE1=(576, 577),
             ONESR=(577, 1089), plmaskrow=(1089, 1349))


def _host_prep(inputs):
    f32 = np.float32
    bf16 = ml_dtypes.bfloat16
    fp8 = ml_dtypes.float8_e4m3  # == mybir.dt.np(dt.float8e4)
    W1 = np.asarray(inputs['Wc1'], f32)
    W2 = np.asarray(inputs['Wc2'], f32)
    W3 = np.asarray(inputs['Wc3'], f32)[0, :, 0, 0]
    b1 = np.asarray(inputs['bc1'], f32)
    b2 = np.asarray(inputs['bc2'], f32)
    b3 = float(np.asarray(inputs['bc3'], f32)[0])
    Wp = np.asarray(inputs['Wp'], f32)
    bp = np.asarray(inputs['bp'], f32)
    Wl, Wr = Wp[:, :DN], Wp[:, DN:]
    WN = np.asarray(inputs['WN'], f32)
    bN = np.asarray(inputs['bN'], f32)
    WE = np.asarray(inputs['WE'], f32)
    bE = np.asarray(inputs['bE'], f32)

    A = W1.sum(axis=3)
    Wv = W1.sum(axis=2)
    cU = np.einsum('oidr,i->od', W1, bp)

    # ---- shared bf16 pack pieces (WpT slot filled per-core) ----
    bpk0 = np.zeros((128, 356), bf16)

    def bput(pk, name, arr):
        a, b_ = _BPK[name]
        pk[:arr.shape[0], a:b_] = arr.astype(bf16)

    bput(bpk0, 'WNT', np.ascontiguousarray(WN.T))
    W2P0 = np.zeros((128, 96), f32)
    W2P1 = np.zeros((128, 96), f32)
    for dr in range(3):
        for j in range(3):
            W2P0[32 * j:32 * j + 32, 32 * dr:32 * dr + 32] = W2[:, :, j, dr].T
        for j in range(1, 4):
            W2P1[32 * j:32 * j + 32, 32 * dr:32 * dr + 32] = W2[:, :, j - 1, dr].T
    W2cat = np.zeros((128, 192), f32)
    for dr in range(3):
        W2cat[:, 64 * dr:64 * dr + 32] = W2P0[:, 32 * dr:32 * dr + 32]
        W2cat[:, 64 * dr + 32:64 * dr + 64] = W2P1[:, 32 * dr:32 * dr + 32]
    bput(bpk0, 'W2cat', W2cat)
    W3sel = np.zeros((128, 4), f32)
    for j in range(4):
        W3sel[32 * j:32 * j + 32, j] = W3
    bput(bpk0, 'W3selb', W3sel)

    # ---- shared 32-row f32 pieces ----
    def pack3(M):  # (co, ci, dl) -> [32, 96] of [ci, co] blocks
        out = np.zeros((32, 96), f32)
        for dl in range(3):
            out[:, 32 * dl:32 * dl + 32] = M[:, :, dl].T
        return out

    cpk32_0 = np.zeros((32, 1248), f32)

    def c32put(name, arr):
        a, b_ = _CPK32[name]
        cpk32_0[:arr.shape[0], a:b_] = arr

    c32put('UW', pack3(A))
    c32put('A0W', pack3(W1[:, :, :, 1:].sum(axis=3)))
    c32put('A511W', pack3(W1[:, :, :, :2].sum(axis=3)))

    W1c0 = np.zeros((32, 192), f32)
    W1c511 = np.zeros((32, 192), f32)
    for dl in range(3):
        for t, dr in enumerate((1, 2)):
            W1c0[:, 32 * (2 * dl + t):32 * (2 * dl + t) + 32] = W1[:, :, dl, dr].T
        for t, dr in enumerate((0, 1)):
            W1c511[:, 32 * (2 * dl + t):32 * (2 * dl + t) + 32] = W1[:, :, dl, dr].T
    c32put('W1c0', W1c0)
    c32put('W1c511', W1c511)

    VW = np.zeros((32, 384), f32)
    for dr in range(3):
        blk = Wv[:, :, dr].T
        for j in range(4):
            VW[:, 128 * dr + 32 * j:128 * dr + 32 * j + 32] = blk
    c32put('VW', VW)
    X0P = np.zeros((32, 96), f32)
    X2P = np.zeros((32, 96), f32)
    for dr in range(3):
        X0P[:, 32 * dr:32 * dr + 32] = W1[:, :, 0, dr].T
        X2P[:, 32 * dr:32 * dr + 32] = W1[:, :, 2, dr].T
    c32put('X0P', X0P)
    c32put('X2P', X2P)

    # ---- shared 1-row f32 pieces (VCfirst/VCqlast flag-baked per core) ----
    c0c = np.zeros((1, 96), f32)
    c511c = np.zeros((1, 96), f32)
    for dl in range(3):
        c0c[0, 32 * dl:32 * dl + 32] = np.einsum('oid,i->o', W1[:, :, dl, 1:], bp)
        c511c[0, 32 * dl:32 * dl + 32] = np.einsum('oid,i->o', W1[:, :, dl, :2], bp)
    c0c[0, 32:64] += b1
    c511c[0, 32:64] += b1
    vc = cU.sum(axis=1) + b1
    VC = np.tile(vc, 4).reshape(1, 128).astype(f32)

    # int4 edge quantization: edge ~= S4 * (q - 7.5); the affine folds exactly
    # into the (bf16-rounded) scaled weights and the GNN bias.
    S4 = np.float32(1.0 / 3.0)
    WEsb = np.ascontiguousarray((WE * S4).T).astype(bf16)      # (64, 128)
    boff = -7.5 * WEsb.astype(f32).sum(axis=0)                 # (128,)
    sh = {'wes': WEsb}

    lig_nf = np.asarray(inputs['ligand_node_features'], f32)
    lig_ef = np.asarray(inputs['ligand_edge_features'], f32)
    rec_nf = np.asarray(inputs['receptor_node_features'], f32)
    rec_ef = np.asarray(inputs['receptor_edge_features'], f32)

    maps = []
    for core in range(8):
        b, h = core // 2, core % 2
        lo = 256 * h - 2
        m = dict(sh)

        # even core: receptor GNN; odd core: ligand GNN
        if h == 0:
            nf, ef, Wpp = rec_nf[b], rec_ef[b], Wr
        else:
            nf, ef, Wpp = lig_nf[b], lig_ef[b], Wl
        m['nodeT'] = np.ascontiguousarray(nf.T).astype(fp8)
        q = np.clip(np.round(ef.reshape(PN, DE).T / S4 + 7.5), 0,
                    15).astype(np.uint8)                       # (64, PN)
        m['edge4'] = np.ascontiguousarray(q[:, 0::2] | (q[:, 1::2] << 4))

        bpk = bpk0.copy()
        bput(bpk, 'WpT', np.ascontiguousarray(Wpp.T))
        m['bpk'] = bpk

        cpk128 = np.zeros((128, 39), f32)

        def c128put(name, arr):
            a, b_ = _CPK128[name]
            cpk128[:arr.shape[0], a:b_] = arr

        c128put('gnnbias', (bN + bE + boff).reshape(DN, 1))
        c128put('WpT16', np.ascontiguousarray((Wpp / 16.0).T)[:, 0:32])
        c128put('bc2rep', np.tile(b2, 4).reshape(128, 1))
        c128put('b3vec', np.full((128, 1), b3, f32))
        flag0 = 1.0 if h == 0 else 0.0
        flag1 = 1.0 if h == 1 else 0.0
        c128put('f0col', np.full((128, 1), flag0, f32))
        c128put('f1col', np.full((128, 1), flag1, f32))
        rmP0 = np.ones((128, 1), f32)
        rmQ63 = np.ones((128, 1), f32)
        for j in range(4):
            if not (0 <= 256 * h + (j - 1) < L):
                rmP0[32 * j:32 * j + 32] = 0.0
            if not (0 <= 256 * h + (253 + j) < L):
                rmQ63[32 * j:32 * j + 32] = 0.0
        c128put('rmP0', rmP0)
        c128put('rmQ63', rmQ63)
        m['cpk128'] = cpk128

        # cpk32 rides the pair AllGather: even core ships rows 0:16, odd 16:32
        m['cpk32h'] = np.ascontiguousarray(cpk32_0[16 * h:16 * h + 16])

        cpk1 = np.zeros((1, 1349), f32)

        def c1put(name, arr):
            a, b_ = _CPK1[name]
            cpk1[:, a:b_] = arr

        c1put('c0const', c0c)
        c1put('c511const', c511c)
        VCfirst, VCqlast = VC.copy(), VC.copy()
        VCfirst[0, 32:64] -= flag0 * cU[:, 0]
        VCqlast[0, 64:96] -= flag1 * cU[:, 2]
        c1put('VC', VC)
        c1put('VCfirst', VCfirst)
        c1put('VCqlast', VCqlast)
        c1put('ONE1', np.ones((1, 1), f32))
        c1put('ONESR', np.ones((1, 512), f32))
        plmask = np.array([1.0 if 0 <= lo + i < L else 0.0 for i in range(260)],
                          f32)
        c1put('plmaskrow', plmask.reshape(1, 260))
        m['cpk1'] = cpk1
        maps.append(m)
    return maps


def _build_program():
    import concourse.bacc as bacc
    import concourse.mybir as mybir
    from concourse.tile import TileContext

    dt = mybir.dt
    f32, bf16, fp8, u8 = dt.float32, dt.bfloat16, dt.float8e4, dt.uint8
    AF = mybir.ActivationFunctionType
    ALU = mybir.AluOpType

    nc = bacc.Bacc("TRN2", target_bir_lowering=False, debug=False, num_devices=8)

    def din(name, shape, dtype=f32):
        return nc.dram_tensor(name, list(shape), dtype, kind="ExternalInput")

    nodeTd = din("nodeT", (128, NN), fp8)
    edge4d = din("edge4", (64, PN // 2), dt.uint8)
    wesd = din("wes", (64, 128), bf16)
    bpkd = din("bpk", (128, 356), bf16)
    cpk128d = din("cpk128", (128, 39))
    cpk32hd = din("cpk32h", (16, 1248))
    cpk1d = din("cpk1", (1, 1349))
    out = nc.dram_tensor("out", [512, 256], u8, kind="ExternalOutput")

    with TileContext(nc) as tc:
        with tc.tile_pool(name="const", bufs=1) as cpool, \
             tc.tile_pool(name="dram", bufs=1, space="DRAM") as dpool:
            WETb_s = cpool.tile([128, 128], bf16, tag="wes")
            nc.sync.dma_start(out=WETb_s[0:64, :], in_=wesd[:])
            # int4 edge unpack: u8 bit ops, then u8 -> bf16 convert
            et8 = cpool.tile([128, PN // 2], dt.uint8, tag="et8")
            nc.sync.dma_start(out=et8[0:64, :], in_=edge4d[:])
            etu = cpool.tile([128, PN], dt.uint8, tag="etu")
            etb = cpool.tile([128, PN], bf16, tag="etb")
            etur = etu[0:64, :].rearrange("p (j t) -> p j t", t=2)
            nc.vector.tensor_scalar(etur[:, :, 0], et8[0:64, :], 15, None,
                                    ALU.bitwise_and)
            nc.vector.tensor_scalar(etur[:, :, 1], et8[0:64, :], 4, None,
                                    ALU.logical_shift_right)
            nc.scalar.activation(etb[0:64, :], etu[0:64, :], AF.Copy)
            bpk_s = cpool.tile([128, 356], bf16, tag="bpk")
            nc.sync.dma_start(out=bpk_s[:], in_=bpkd[:])
            cpk128_s = cpool.tile([128, 39], f32, tag="cpk128")
            nc.sync.dma_start(out=cpk128_s[:], in_=cpk128d[:])
            cpk32_s = cpool.tile([128, 1248], f32, tag="cpk32")
            cpk1_s = cpool.tile([128, 1349], f32, tag="cpk1")
            nc.sync.dma_start(out=cpk1_s[0:1, :], in_=cpk1d[:])
            nodeTb_s = cpool.tile([128, NN], bf16, tag="nodeTb")
            nc.gpsimd.dma_start(out=nodeTb_s[:], in_=nodeTd[:])

            def bsl(name, rows=128):
                a, b_ = _BPK[name]
                return bpk_s[0:rows, a:b_]

            def c128sl(name, rows=128):
                a, b_ = _CPK128[name]
                return cpk128_s[0:rows, a:b_]

            def c32sl(name, rows=32):
                a, b_ = _CPK32[name]
                return cpk32_s[0:rows, a:b_]

            def c1sl(name):
                a, b_ = _CPK1[name]
                return cpk1_s[0:1, a:b_]

            WNT_s = bsl('WNT')
            WpT_s = bsl('WpT')
            W2cat_s = bsl('W2cat')
            W3sel_s = bsl('W3selb')
            gnnbias_s = c128sl('gnnbias')
            WpT16_s = c128sl('WpT16')
            bc2rep_s, b3vec_s = c128sl('bc2rep'), c128sl('b3vec')
            rmP0_s, rmQ63_s = c128sl('rmP0'), c128sl('rmQ63')
            f0col_s, f1col_s = c128sl('f0col'), c128sl('f1col')
            UW_s = c32sl('UW')
            W1c0_s, W1c511_s = c32sl('W1c0'), c32sl('W1c511')
            VW_s = c32sl('VW')
            X0P_s, X2P_s = c32sl('X0P'), c32sl('X2P')
            c0c_s, c511c_s = c1sl('c0const'), c1sl('c511const')
            VC_s, VCf_s, VCq_s = c1sl('VC'), c1sl('VCfirst'), c1sl('VCqlast')
            ONE1_s, ONESR_s = c1sl('ONE1'), c1sl('ONESR')
            plmaskrow_s = c1sl('plmaskrow')

            S_t = cpool.tile([128, NN], f32)
            projm = cpool.tile([128, 512], f32)   # rows 0:32: own projection
            plx = cpool.tile([128, 516], f32)     # rows 0:32: [0,0, pl, 0,0]
            selt = cpool.tile([128, 260], f32)
            plzA = cpool.tile([128, 260], f32)    # rows 0-31 plz, row 32 mask
            przA = cpool.tile([128, 514], f32)
            U_sb = cpool.tile([128, 260], f32)
            Uc0_sb = cpool.tile([128, 260], f32)
            Uc511_sb = cpool.tile([128, 260], f32)
            A0AUG = cpool.tile([128, 96], f32)
            A511AUG = cpool.tile([128, 96], f32)
            VWf_t = cpool.tile([128, 384], f32)
            VWq_t = cpool.tile([128, 384], f32)
            Xf_t = cpool.tile([128, 96], f32)
            V_rep = cpool.tile([128, 512], f32)
            V_first = cpool.tile([128, 512], f32)
            V_qlast = cpool.tile([128, 512], f32)
            uP = cpool.tile([128, 64], f32, tag="uP")
            uQ = cpool.tile([128, 64], f32, tag="uQ")
            uc0P = cpool.tile([128, 64], f32, tag="uc0P")
            uc0Q = cpool.tile([128, 64], f32, tag="uc0Q")
            uc511P = cpool.tile([128, 64], f32, tag="uc511P")
            uc511Q = cpool.tile([128, 64], f32, tag="uc511Q")

            # ================= GNN phase (own molecule only) =================
            with tc.tile_pool(name="gnn", bufs=4) as gpool, \
                 tc.tile_pool(name="gpsum", bufs=3, space="PSUM") as gpsum, \
                 tc.tile_pool(name="spsum", bufs=1, space="PSUM") as spsum:

                for c in range(PN // (CH * KNB)):
                    c0 = c * CH * KNB
                    hz = gpsum.tile([128, CH * KNB], f32, tag="hz")
                    for q in range(CH * KNB // 512):
                        nc.tensor.matmul(
                            hz[:, q * 512:(q + 1) * 512],
                            WETb_s[0:64, :],
                            etb[0:64, c0 + q * 512:c0 + (q + 1) * 512],
                            start=True, stop=False)
                        rhs = nodeTb_s[:, c * CH + q * 32:c * CH + (q + 1) * 32]
                        rhs = rhs.unsqueeze(2).broadcast_to([128, 32, KNB])
                        nc.tensor.matmul(
                            hz[:, q * 512:(q + 1) * 512],
                            WNT_s, rhs,
                            start=False, stop=True)
                    zt = gpool.tile([128, CH * KNB], bf16, tag="zt")
                    nc.scalar.activation(zt[:], hz[:], AF.Tanh, bias=gnnbias_s)
                    ztr = zt[:].rearrange("p (n k) -> p n k", k=KNB)
                    nc.vector.reduce_sum(
                        S_t[:, c * CH:(c + 1) * CH], ztr,
                        axis=mybir.AxisListType.X)

                # ---- HOPI projection of own molecule ----
                pp = spsum.tile([128, 512], f32, tag="sp")
                nc.tensor.matmul(pp[0:32, 0:NN], WpT_s, nodeTb_s[:],
                                 start=True, stop=False)
                nc.tensor.matmul(pp[0:32, 0:NN], WpT16_s, S_t[:],
                                 start=False, stop=True)
                nc.scalar.activation(projm[0:32, :], pp[0:32, 0:NN], AF.Copy)

                # ---- pairwise exchange: cols 0:1024 carry the (32,512) proj
                # viewed as (16,1024); cols 1024:2272 carry this core's half of
                # cpk32. Gathered rows 0:16 = even core (pr + cpk32[0:16]),
                # rows 16:32 = odd core (pl + cpk32[16:32]). ----
                cin = dpool.tile([16, 2272], f32, tag="cin")
                cout = dpool.tile([32, 2272], f32, tag="cout")
                nc.gpsimd.dma_start(out=cin[:, 0:512], in_=projm[0:16, :])
                nc.gpsimd.dma_start(out=cin[:, 512:1024], in_=projm[16:32, :])
                nc.gpsimd.dma_start(out=cin[:, 1024:2272], in_=cpk32hd[:])
                nc.gpsimd.collective_compute(
                    "AllGather", ALU.bypass,
                    replica_groups=[[0, 1], [2, 3], [4, 5], [6, 7]],
                    ins=[cin.opt()], outs=[cout.opt()])
                nc.vector.memset(przA[0:32, 0:1], 0.0)
                nc.vector.memset(przA[0:32, 513:514], 0.0)
                nc.sync.dma_start(out=przA[0:16, 1:513], in_=cout[0:16, 0:512])
                nc.sync.dma_start(out=przA[16:32, 1:513], in_=cout[0:16, 512:1024])
                nc.vector.memset(plx[0:32, 0:2], 0.0)
                nc.vector.memset(plx[0:32, 514:516], 0.0)
                nc.sync.dma_start(out=plx[0:16, 2:514], in_=cout[16:32, 0:512])
                nc.sync.dma_start(out=plx[16:32, 2:514], in_=cout[16:32, 512:1024])
                nc.sync.dma_start(out=cpk32_s[0:32, :], in_=cout[:, 1024:2272])

                # ---- plzA = f0*plx[:,0:260] + f1*plx[:,256:516]; row 32 mask ----
                f0c32, f1c32 = c128sl('f0col', rows=32), c128sl('f1col', rows=32)
                nc.vector.tensor_scalar(plzA[0:32, :], plx[0:32, 0:260],
                                        f0c32, None, ALU.mult)
                nc.vector.tensor_scalar(selt[0:32, :], plx[0:32, 256:516],
                                        f1c32, None, ALU.mult)
                nc.vector.tensor_add(plzA[0:32, :], plzA[0:32, :], selt[0:32, :])
                nc.sync.dma_start(out=plzA[32:33, :], in_=plmaskrow_s)

                # ---- V weight variants from flags ----
                nc.scalar.activation(VWf_t[0:32, :], VW_s, AF.Copy)
                nc.scalar.activation(VWq_t[0:32, :], VW_s, AF.Copy)
                nc.vector.tensor_scalar(Xf_t[0:32, :], X0P_s, f0c32, None,
                                        ALU.mult)
                for dr in range(3):
                    nc.vector.tensor_sub(
                        VWf_t[0:32, 128 * dr + 32:128 * dr + 64],
                        VWf_t[0:32, 128 * dr + 32:128 * dr + 64],
                        Xf_t[0:32, 32 * dr:32 * dr + 32])
                nc.vector.tensor_scalar(Xf_t[0:32, :], X2P_s, f1c32, None,
                                        ALU.mult)
                for dr in range(3):
                    nc.vector.tensor_sub(
                        VWq_t[0:32, 128 * dr + 64:128 * dr + 96],
                        VWq_t[0:32, 128 * dr + 64:128 * dr + 96],
                        Xf_t[0:32, 32 * dr:32 * dr + 32])

                # ---- U ----
                up = spsum.tile([128, 512], f32, tag="sp")
                for dl in range(3):
                    nc.tensor.matmul(up[0:32, 0:258],
                                     UW_s[0:32, 32 * dl:32 * dl + 32],
                                     plzA[0:32, dl:dl + 258],
                                     start=(dl == 0), stop=(dl == 2))
                nc.scalar.activation(U_sb[0:32, 0:258], up[0:32, 0:258], AF.Copy)

                # ---- c0 / c511 rows ----
                nc.sync.dma_start(out=A0AUG[0:32, :],
                                  in_=cout[:, 1024 + 96:1024 + 192])
                nc.sync.dma_start(out=A511AUG[0:32, :],
                                  in_=cout[:, 1024 + 192:1024 + 288])
                for which, (W1c_s, cc_s, dst) in enumerate(
                        ((W1c0_s, c0c_s, A0AUG), (W1c511_s, c511c_s, A511AUG))):
                    cp = spsum.tile([128, 512], f32, tag="sp")
                    for dl in range(3):
                        for t in range(2):
                            col = (1 + t) if which == 0 else (511 + t)
                            nc.tensor.matmul(
                                cp[0:1, 32 * dl:32 * dl + 32],
                                przA[0:32, col:col + 1],
                                W1c_s[0:32, 32 * (2 * dl + t):32 * (2 * dl + t) + 32],
                                start=(t == 0), stop=False)
                        nc.tensor.matmul(
                            cp[0:1, 32 * dl:32 * dl + 32],
                            ONE1_s,
                            cc_s[0:1, 32 * dl:32 * dl + 32],
                            start=False, stop=True)
                    nc.scalar.activation(dst[32:33, 0:96], cp[0:1, 0:96], AF.Copy)

                # ---- Ucol0 / Ucol511 ----
                for AUG, dstu in ((A0AUG, Uc0_sb), (A511AUG, Uc511_sb)):
                    ucp = spsum.tile([128, 512], f32, tag="sp")
                    for dl in range(3):
                        nc.tensor.matmul(ucp[0:32, 0:258],
                                         AUG[0:33, 32 * dl:32 * dl + 32],
                                         plzA[0:33, dl:dl + 258],
                                         start=(dl == 0), stop=(dl == 2))
                    nc.scalar.activation(dstu[0:32, 0:258], ucp[0:32, 0:258], AF.Copy)

                # ---- V variants ----
                for VWx, VCx, vt in ((VW_s, VC_s, V_rep),
                                     (VWf_t[0:32, :], VCf_s, V_first),
                                     (VWq_t[0:32, :], VCq_s, V_qlast)):
                    vp = spsum.tile([128, 512], f32, tag="sp")
                    for dr in range(3):
                        nc.tensor.matmul(vp[:, 0:512],
                                         VWx[0:32, 128 * dr:128 * dr + 128],
                                         przA[0:32, dr:dr + 512],
                                         start=(dr == 0), stop=False)
                    nc.tensor.matmul(vp[:, 0:512], VCx, ONESR_s,
                                     start=False, stop=True)
                    nc.scalar.activation(vt[:], vp[:, 0:512], AF.Copy)

                # ---- u relayouts (i = 4s+j for P, 4s+2+j for Q) ----
                for (src, dstP, dstQ) in ((U_sb, uP, uQ), (Uc0_sb, uc0P, uc0Q),
                                          (Uc511_sb, uc511P, uc511Q)):
                    srcr = src[0:32, 0:260].rearrange("c (s f) -> c s f", f=4)
                    for j in range(4):
                        nc.sync.dma_start(out=dstP[32 * j:32 * j + 32, 0:64],
                                          in_=srcr[:, 0:64, j])
                    for j in range(2):
                        nc.sync.dma_start(out=dstQ[32 * j:32 * j + 32, 0:64],
                                          in_=srcr[:, 0:64, 2 + j])
                    for j in range(2, 4):
                        nc.sync.dma_start(out=dstQ[32 * j:32 * j + 32, 0:64],
                                          in_=srcr[:, 1:65, j - 2])
                for (t, col, rm) in ((uP, 0, rmP0_s), (uc0P, 0, rmP0_s),
                                     (uc511P, 0, rmP0_s), (uQ, 63, rmQ63_s),
                                     (uc0Q, 63, rmQ63_s), (uc511Q, 63, rmQ63_s)):
                    nc.vector.tensor_mul(t[:, col:col + 1], t[:, col:col + 1], rm)

            # ================= conv pipeline =================
            with tc.tile_pool(name="x1", bufs=3) as x1pool, \
                 tc.tile_pool(name="x2", bufs=3) as x2pool, \
                 tc.tile_pool(name="osb", bufs=2) as opool, \
                 tc.tile_pool(name="cpsum", bufs=4, space="PSUM") as cpsum, \
                 tc.tile_pool(name="c3ps", bufs=2, space="PSUM") as c3psum:

                for k in range(NSTRIP):
                    x1P = x1pool.tile([128, 8 * 514], bf16, tag="x1P")
                    x1Q = x1pool.tile([128, 8 * 514], bf16, tag="x1Q")
                    for s in range(8):
                        sg = 8 * k + s
                        for (tile_, uu, Vgen, is_edge, rm) in (
                                (x1P, uP, V_first if sg == 0 else V_rep, sg == 0, rmP0_s),
                                (x1Q, uQ, V_qlast if sg == 63 else V_rep, sg == 63, rmQ63_s)):
                            dst = tile_[:, s * 514 + 1:s * 514 + 513]
                            bias_ap = uu[:, sg:sg + 1]
                            if is_edge:
                                nc.scalar.activation(dst, Vgen[:], AF.Relu,
                                                     bias=bias_ap, scale=rm)
                            elif s % 3 == 0:
                                nc.scalar.activation(dst, Vgen[:], AF.Relu, bias=bias_ap)
                            else:
                                nc.vector.tensor_scalar(dst, Vgen[:], bias_ap, 0.0,
                                                        ALU.add, ALU.max)
                    for tile_, ucol0, ucol511 in ((x1P, uc0P, uc511P), (x1Q, uc0Q, uc511Q)):
                        tr = tile_[:].rearrange("p (s c) -> p s c", c=514)
                        nc.vector.memset(tr[:, :, 0], 0.0)
                        nc.vector.memset(tr[:, :, 513], 0.0)
                        nc.vector.tensor_scalar(tr[:, :, 1], ucol0[:, 8 * k:8 * k + 8],
                                                0.0, None, ALU.max)
                        nc.vector.tensor_scalar(tr[:, :, 512], ucol511[:, 8 * k:8 * k + 8],
                                                0.0, None, ALU.max)

                    x2 = x2pool.tile([128, 8 * 512], bf16, tag="x2")
                    for s in range(8):
                        c2 = cpsum.tile([128, 512], f32, tag="c2")
                        for dr in range(3):
                            wcat = W2cat_s[:, 64 * dr:64 * dr + 64]
                            rhsP = x1P[:, s * 514 + dr:s * 514 + dr + 512]
                            rhsQ = x1Q[:, s * 514 + dr:s * 514 + dr + 512]
                            st, sp_ = (dr == 0), (dr == 2)
                            nc.tensor.matmul(c2[0:64, :], wcat, rhsP, start=st, stop=sp_,
                                             tile_position=(0, 0), skip_group_check=True)
                            nc.tensor.matmul(c2[64:128, :], wcat, rhsQ, start=st, stop=sp_,
                                             tile_position=(0, 64), skip_group_check=True)
                        dst2 = x2[:, s * 512:(s + 1) * 512]
                        if s % 3 != 2:
                            nc.scalar.activation(dst2, c2[:], AF.Relu, bias=bc2rep_s)
                        else:
                            nc.vector.tensor_scalar(dst2, c2[:], bc2rep_s, 0.0,
                                                    ALU.add, ALU.max)

                    # conv3: logits transposed onto 128 partitions (r-slab on
                    # partitions, strip-row on free); undone host-side.
                    c3p = c3psum.tile([128, 128], f32, tag="c3")
                    for s in range(8):
                        xc = x2[:, s * 512:(s + 1) * 512]
                        for u in range(4):
                            nc.tensor.matmul(
                                c3p[:, 32 * u + 4 * s:32 * u + 4 * s + 4],
                                xc[:, 128 * u:128 * u + 128],
                                W3sel_s, start=True, stop=True)
                    sgt = opool.tile([128, 128], f32, tag="sgt")
                    nc.scalar.activation(sgt[:], c3p[:], AF.Sigmoid,
                                         bias=b3vec_s)
                    osb = opool.tile([128, 128], u8, tag="osb")
                    nc.vector.tensor_scalar(osb[:], sgt[:], 255.0, None, ALU.mult)
                    # osb[p, 32u+4s+m] = 255*sigmoid(logit[row=4s+m, r=128u+p])
                    osr = osb[:].rearrange("p (u c) -> p u c", c=32)
                    outr = out[:].rearrange("(u p) g -> p u g", p=128)
                    nc.sync.dma_start(out=outr[:, :, 32 * k:32 * k + 32],
                                      in_=osr)

    nc.compile()
    return nc


def kernel(**inputs):
    from concourse.bass_utils import run_bass_kernel_spmd
    if "nc" not in _CACHE:
        _CACHE["nc"] = _build_program()
    nc = _CACHE["nc"]
    maps = _host_prep(inputs)
    res = run_bass_kernel_spmd(nc, maps, core_ids=list(range(8)))
    _CACHE["last_result"] = res
    full = np.zeros((B, L, R), np.float32)
    for core in range(8):
        b, h = core // 2, core % 2
        full[b, 256 * h:256 * h + 256, :] = \
            (res.results[core]["out"].astype(np.float32) / 255.0).T
    return full
